# revision 14
# baseline (speedup 1.0000x reference)
"""Trainium2 Bass kernel for nn_BasicBlock_63496796504726
(sparse 3x3x3 conv -> BN -> ReLU -> sparse conv -> BN -> +residual -> ReLU).

Sharding: out-voxel rows sharded across 8 NeuronCores (32768 rows each);
x replicated per core (fp16). Per core, kernel-map pairs whose output row
falls in its shard are processed as:
  dma_gather(transpose) per (k, in-bucket) group  -> gathered^T [C, G] fp16
  matmul (lhsT = gathered^T tile stationary, rhs = W[k])
                                                  -> contrib [128, C] psum
  copy/cast fp16 wrapped                          -> contrib [128, g, C]
  dma_scatter_add into fp16 DRAM accumulators (duplicate-free per call via
  occurrence classes; same-accumulator calls are serialized by Tile)
BN: merge accumulators, per-channel sums via ones-matmul, AllReduce [1,2C]
stats, scale/shift broadcast by rank-1 matmul, ReLU. x1 shards AllGathered
for conv2's gathers. Residual + ReLU in fp32 at the end.
"""
import sys

sys.path.insert(0, "/opt/trn_rl_repo")

import numpy as np

# problem constants (shrinkable for simulator tests)
N = 262144
C = 128
K = 27
M = 131072
NCORES = 8
SHARD = N // NCORES
BUCKET = 32768               # dma_gather int16 index window
EPS = 1e-5
NCLASS = 6                   # duplicate-occurrence classes per group
NACC = 8                     # independent accumulators per conv


def _nbucket():
    return (N + BUCKET - 1) // BUCKET


# ---------------------------------------------------------------- host prep

def _host_prep(in_maps, out_maps):
    NB = _nbucket()
    kf = np.repeat(np.arange(K), M)
    inf_ = in_maps.ravel().astype(np.int64)
    outf = out_maps.ravel().astype(np.int64)
    core = outf // SHARD
    bucket = inf_ // BUCKET
    out_local = outf % SHARD
    gloc = inf_ % BUCKET

    half = out_local // (SHARD // 2)
    order = np.lexsort((out_local, half, bucket, kf, core))
    sc, sk, sb = core[order], kf[order], bucket[order]
    sr, sg = out_local[order], gloc[order]
    sh = half[order]

    # occurrence rank within (core,k,bucket,row)
    gk = ((sc * K + sk) * NB + sb) * SHARD + sr
    new = np.empty(len(gk), bool); new[0] = True
    new[1:] = gk[1:] != gk[:-1]
    st = np.flatnonzero(new)
    occ = np.arange(len(gk)) - np.repeat(st, np.diff(np.append(st, len(gk))))
    if occ.max() >= NCLASS:
        raise RuntimeError(f"max dup occurrence {occ.max()} >= NCLASS={NCLASS}")

    counts = np.zeros((NCORES, K, NB, 2, NCLASS), np.int64)
    np.add.at(counts, (sc, sk, sb, sh, occ), 1)
    caps = counts.max(axis=0)
    caps = ((caps + 127) // 128) * 128          # 0 stays 0
    ccols = caps // 128
    gcols = ccols.sum(axis=(2, 3))
    TOTCOL = int(gcols.sum())
    TOT = TOTCOL * 128

    gofs = np.zeros((K, NB), np.int64)
    cofs = np.zeros((K, NB, 2, NCLASS), np.int64)
    acc = 0
    for k in range(K):
        for b in range(NB):
            gofs[k, b] = acc
            for h in range(2):
                for c in range(NCLASS):
                    cofs[k, b, h, c] = acc
                    acc += ccols[k, b, h, c]
    assert acc == TOTCOL

    HS = SHARD // 2
    gstr = np.zeros((NCORES, TOT), np.int16)
    sstr = np.full((NCORES, TOT), HS, np.int16)      # pads -> dump row
    pk = (((sc * K + sk) * NB + sb) * 2 + sh) * NCLASS + occ
    po = np.lexsort((np.arange(len(pk)), pk))
    pks = pk[po]
    npk = np.empty(len(pks), bool); npk[0] = True
    npk[1:] = pks[1:] != pks[:-1]
    ps = np.flatnonzero(npk)
    rank = np.arange(len(pks)) - np.repeat(ps, np.diff(np.append(ps, len(pks))))
    pos = cofs[sk[po], sb[po], sh[po], occ[po]] * 128 + rank
    gstr[sc[po], pos] = sg[po].astype(np.int16)
    sstr[sc[po], pos] = (sr[po] % HS).astype(np.int16)

    meta = dict(ccols=ccols, gcols=gcols, gofs=gofs, cofs=cofs,
                TOTCOL=TOTCOL, TOT=TOT, NB=NB)
    return gstr, sstr, meta


def _wrap16(stream):
    n = stream.shape[0]
    w = stream.reshape(n // 16, 16).T
    return np.tile(w, (8, 1)).copy()


# ---------------------------------------------------------------- builder

def _build(meta):
    import concourse.bacc as bacc
    import concourse.mybir as mybir
    import concourse.tile as tile

    f16, f32 = mybir.dt.float16, mybir.dt.float32
    i16 = mybir.dt.int16
    ccols, gcols = meta["ccols"], meta["gcols"]
    gofs, cofs = meta["gofs"], meta["cofs"]
    TOT = meta["TOT"]
    NB = meta["NB"]
    HS = SHARD // 2
    HS1 = HS + 1
    TROWS = SHARD // 128
    HROWS = HS // 128
    ZCH = min(32, HROWS)
    AACC = NACC // 2

    nc = bacc.Bacc("TRN2", target_bir_lowering=False, debug=False,
                   num_devices=NCORES)

    x16 = nc.dram_tensor("x16", [N, C], f16, kind="ExternalInput").ap()
    w1 = nc.dram_tensor("w1", [C, K * C], f16, kind="ExternalInput").ap()
    w2 = nc.dram_tensor("w2", [C, K * C], f16, kind="ExternalInput").ap()
    gidx = nc.dram_tensor("gidx", [128, TOT // 16], i16, kind="ExternalInput").ap()
    sidx = nc.dram_tensor("sidx", [128, TOT // 16], i16, kind="ExternalInput").ap()
    bnp = nc.dram_tensor("bnp", [1, 4 * C], f32, kind="ExternalInput").ap()
    xres = nc.dram_tensor("xres", [SHARD, C], f16, kind="ExternalInput").ap()
    y = nc.dram_tensor("y", [SHARD, C], f32, kind="ExternalOutput").ap()

    accs = [[[nc.dram_tensor(f"acc{cv}_{h}_{a}", [HS1, C], f16)
              for a in range(AACC)] for h in range(2)] for cv in range(2)]
    x1_shard = nc.dram_tensor("x1_shard", [SHARD, C], f16)
    x1_full = nc.dram_tensor("x1_full", [NCORES * SHARD, C], f16)
    stats_in = nc.dram_tensor("stats_in", [1, 2 * C], f32)
    stats_out = nc.dram_tensor("stats_out", [1, 2 * C], f32)

    with tile.TileContext(nc) as tc:
        with tc.tile_pool(name="persist", bufs=1) as pp:
            w_t = []
            for i in range(2):
                wti = pp.tile([C, K * C], f16, tag=f"w{i}")
                w_t.append(wti)
            nc.sync.dma_start(out=w_t[0][:], in_=w1[:])
            nc.sync.dma_start(out=w_t[1][:], in_=w2[:])
            bnp_t = pp.tile([1, 4 * C], f32)
            nc.sync.dma_start(out=bnp_t[:], in_=bnp[:])
            ones16 = pp.tile([128, 1], f16)
            nc.vector.memset(ones16[:], 1.0)
            onesr = pp.tile([1, 128], f32)
            nc.vector.memset(onesr[:], 1.0)
            zt = pp.tile([128, ZCH, C], f16)
            nc.vector.memset(zt[:], 0.0)

            for cv in range(2):
                for h in range(2):
                    for a in range(AACC):
                        av = accs[cv][h][a].ap()[0:HS, :].rearrange(
                            "(t p) c -> p t c", p=128)
                        for j in range(HROWS // ZCH):
                            nc.sync.dma_start(
                                out=av[:, j * ZCH:(j + 1) * ZCH, :], in_=zt[:])
                        nc.sync.dma_start(out=accs[cv][h][a].ap()[HS:HS1, :],
                                          in_=zt[0:1, 0, :])

            def conv(cv, src_ap):
                wt = w_t[cv]
                with tc.tile_pool(name=f"conv{cv}", bufs=4) as cp, \
                     tc.tile_pool(name=f"cps{cv}", bufs=8, space="PSUM") as cps, \
                     tc.tile_pool(name=f"cix{cv}", bufs=4) as cip:
                    call_no = 0
                    for k in range(K):
                        for b in range(NB):
                            g0 = int(gofs[k, b]); gc = int(gcols[k, b])
                            if gc == 0:
                                continue
                            gi = cip.tile([128, gc * 8], i16, tag="gi")
                            si = cip.tile([128, gc * 8], i16, tag="si")
                            nc.sync.dma_start(out=gi[:],
                                              in_=gidx[:, g0 * 8:(g0 + gc) * 8])
                            nc.sync.dma_start(out=si[:],
                                              in_=sidx[:, g0 * 8:(g0 + gc) * 8])
                            gt = cp.tile([C, 1, gc * 128], f16, tag="gt")
                            nc.gpsimd.dma_gather(
                                out_ap=gt[:],
                                in_ap=src_ap[b * BUCKET:(b + 1) * BUCKET, :],
                                idxs_ap=gi[:],
                                num_idxs=gc * 128,
                                num_idxs_reg=gc * 128,
                                elem_size=C,
                                transpose=True,
                            )
                            contrib = cp.tile([128, gc, C], f16, tag="ct")
                            for gidx_cp, t0 in enumerate(range(0, gc, 4)):
                                tw = min(4, gc - t0)
                                pst = cps.tile([128, 4 * C], f32, tag="ps")
                                for t in range(t0, t0 + tw):
                                    nc.tensor.matmul(
                                        pst[:, (t - t0) * C:(t - t0 + 1) * C],
                                        lhsT=gt[:, 0, t * 128:(t + 1) * 128],
                                        rhs=wt[:, k * C:(k + 1) * C],
                                        start=True, stop=True)
                                eng = nc.scalar if (gidx_cp % 2 == 0) else nc.vector
                                if gidx_cp % 2 == 0:
                                    nc.scalar.copy(
                                        contrib[:, t0:t0 + tw, :],
                                        pst[:, 0:tw * C].rearrange(
                                            "p (t c) -> p t c", c=C))
                                else:
                                    nc.vector.tensor_copy(
                                        contrib[:, t0:t0 + tw, :],
                                        pst[:, 0:tw * C].rearrange(
                                            "p (t c) -> p t c", c=C))
                            for hh in range(2):
                              for cl in range(NCLASS):
                                cc = int(ccols[k, b, hh, cl])
                                if cc == 0:
                                    continue
                                c0 = int(cofs[k, b, hh, cl]) - g0
                                acc_t = accs[cv][hh][call_no % AACC]
                                call_no += 1
                                nc.gpsimd.dma_scatter_add(
                                    out_ap=acc_t.ap(),
                                    in_ap=contrib[:, c0:c0 + cc, :],
                                    idxs_ap=si[:, c0 * 8:(c0 + cc) * 8],
                                    num_idxs=cc * 128,
                                    num_idxs_reg=cc * 128,
                                    elem_size=C)

            def merge_bn(cv, out_cb):
                with tc.tile_pool(name=f"bn{cv}", bufs=1) as bp, \
                     tc.tile_pool(name=f"bna{cv}", bufs=4) as bap, \
                     tc.tile_pool(name=f"bnp{cv}", bufs=1, space="PSUM") as bps:
                    msb = bp.tile([128, TROWS, C], f16, tag="msb")
                    pss = bps.tile([1, 512], f32, tag="pss")
                    psq = bps.tile([1, 512], f32, tag="psq")
                    nch = TROWS // 4
                    for j in range(nch):
                        hh = j // (nch // 2)
                        jj = j % (nch // 2)
                        mt = bap.tile([128, 4, C], f16, tag="mt")
                        av0 = accs[cv][hh][0].ap()[0:HS, :].rearrange(
                            "(t p) c -> p t c", p=128)
                        nc.sync.dma_start(out=mt[:], in_=av0[:, jj * 4:(jj + 1) * 4, :])
                        for a in range(1, AACC):
                            at = bap.tile([128, 4, C], f16, tag="at")
                            av = accs[cv][hh][a].ap()[0:HS, :].rearrange(
                                "(t p) c -> p t c", p=128)
                            nc.sync.dma_start(out=at[:],
                                              in_=av[:, jj * 4:(jj + 1) * 4, :])
                            nc.vector.tensor_add(mt[:], mt[:], at[:])
                        nc.vector.tensor_copy(msb[:, j * 4:(j + 1) * 4, :], mt[:])
                        sq = bap.tile([128, 4, C], f16, tag="sq")
                        nc.vector.tensor_mul(sq[:], mt[:], mt[:])
                        nc.tensor.matmul(
                            pss[:], lhsT=ones16[:],
                            rhs=mt[:].rearrange("p t c -> p (t c)"),
                            start=(j == 0), stop=(j == nch - 1))
                        nc.tensor.matmul(
                            psq[:], lhsT=ones16[:],
                            rhs=sq[:].rearrange("p t c -> p (t c)"),
                            start=(j == 0), stop=(j == nch - 1))
                    pssb = bp.tile([1, 512], f32, tag="pssb")
                    psqb = bp.tile([1, 512], f32, tag="psqb")
                    nc.vector.tensor_copy(pssb[:], pss[:])
                    nc.vector.tensor_copy(psqb[:], psq[:])
                    stt = bp.tile([1, 2 * C], f32, tag="stt")
                    nc.vector.tensor_add(stt[:, 0:C], pssb[:, 0:C], pssb[:, C:2 * C])
                    nc.vector.tensor_add(stt[:, 0:C], stt[:, 0:C], pssb[:, 2 * C:3 * C])
                    nc.vector.tensor_add(stt[:, 0:C], stt[:, 0:C], pssb[:, 3 * C:4 * C])
                    nc.vector.tensor_add(stt[:, C:2 * C], psqb[:, 0:C], psqb[:, C:2 * C])
                    nc.vector.tensor_add(stt[:, C:2 * C], stt[:, C:2 * C],
                                         psqb[:, 2 * C:3 * C])
                    nc.vector.tensor_add(stt[:, C:2 * C], stt[:, C:2 * C],
                                         psqb[:, 3 * C:4 * C])
                    nc.sync.dma_start(out=stats_in.ap(), in_=stt[:])
                    nc.gpsimd.collective_compute(
                        "AllReduce", mybir.AluOpType.add,
                        replica_groups=[list(range(NCORES))],
                        ins=[stats_in.ap().opt()],
                        outs=[stats_out.ap().opt()])
                    stg = bp.tile([1, 2 * C], f32, tag="stg")
                    nc.sync.dma_start(out=stg[:], in_=stats_out.ap())
                    mean = bp.tile([1, C], f32, tag="mean")
                    var = bp.tile([1, C], f32, tag="var")
                    inv = bp.tile([1, C], f32, tag="inv")
                    msq = bp.tile([1, C], f32, tag="msq")
                    sc_t = bp.tile([1, C], f32, tag="sct")
                    sh_t = bp.tile([1, C], f32, tag="sht")
                    nc.vector.tensor_scalar_mul(mean[:], stg[:, 0:C], 1.0 / N)
                    nc.vector.tensor_scalar_mul(var[:], stg[:, C:2 * C], 1.0 / N)
                    nc.vector.tensor_mul(msq[:], mean[:], mean[:])
                    nc.vector.tensor_sub(var[:], var[:], msq[:])
                    nc.vector.tensor_scalar_add(var[:], var[:], EPS)
                    nc.scalar.activation(inv[:], var[:],
                                         mybir.ActivationFunctionType.Sqrt)
                    nc.vector.reciprocal(inv[:], inv[:])
                    nc.vector.tensor_mul(sc_t[:], inv[:],
                                         bnp_t[:, 2 * cv * C:(2 * cv + 1) * C])
                    nc.vector.tensor_mul(sh_t[:], mean[:], sc_t[:])
                    nc.vector.tensor_sub(sh_t[:], bnp_t[:, (2 * cv + 1) * C:(2 * cv + 2) * C],
                                         sh_t[:])
                    psb = bps.tile([128, 2 * C], f32, tag="psb")
                    nc.tensor.matmul(psb[:, 0:C], lhsT=onesr[:], rhs=sc_t[:],
                                     start=True, stop=True)
                    nc.tensor.matmul(psb[:, C:2 * C], lhsT=onesr[:], rhs=sh_t[:],
                                     start=True, stop=True)
                    scb = bp.tile([128, C], f16, tag="scb")
                    shb = bp.tile([128, C], f16, tag="shb")
                    nc.vector.tensor_copy(scb[:], psb[:, 0:C])
                    nc.vector.tensor_copy(shb[:], psb[:, C:2 * C])
                    for j in range(nch):
                        nt = bap.tile([128, 4, C], f16, tag="nt")
                        for tt in range(4):
                            nc.vector.tensor_mul(
                                nt[:, tt, :], msb[:, j * 4 + tt, :], scb[:])
                            nc.vector.tensor_add(
                                nt[:, tt, :], nt[:, tt, :], shb[:])
                        out_cb(j, nt, bap)

            conv(0, x16)

            x1v = x1_shard.ap().rearrange("(t p) c -> p t c", p=128)

            def bn1_out(j, nt, bap):
                xt = bap.tile([128, 4, C], f16, tag="xt")
                nc.vector.tensor_scalar_max(xt[:], nt[:], 0.0)
                nc.sync.dma_start(out=x1v[:, j * 4:(j + 1) * 4, :], in_=xt[:])

            merge_bn(0, bn1_out)

            import concourse.mybir as mybir2
            nc.gpsimd.collective_compute(
                "AllGather", mybir2.AluOpType.bypass,
                replica_groups=[list(range(NCORES))],
                ins=[x1_shard.ap().opt()],
                outs=[x1_full.ap().opt()])

            conv(1, x1_full.ap())

            yv = y.rearrange("(t p) c -> p t c", p=128)
            xrv = xres.rearrange("(t p) c -> p t c", p=128)

            def bn2_out(j, nt, bap):
                rt = bap.tile([128, 4, C], f32, tag="rt")
                rt16 = bap.tile([128, 4, C], f16, tag="rt16")
                nc.sync.dma_start(out=rt16[:],
                                  in_=xrv[:, j * 4:(j + 1) * 4, :])
                nc.vector.tensor_copy(rt[:], rt16[:])
                nt32 = bap.tile([128, 4, C], f32, tag="nt32")
                nc.vector.tensor_copy(nt32[:], nt[:])
                nc.vector.tensor_add(nt32[:], nt32[:], rt[:])
                yt = bap.tile([128, 4, C], f32, tag="yt")
                nc.vector.tensor_scalar_max(yt[:], nt32[:], 0.0)
                nc.sync.dma_start(out=yv[:, j * 4:(j + 1) * 4, :], in_=yt[:])

            merge_bn(1, bn2_out)

    nc.compile()
    return nc


_CACHE = {}


def _prepare(feats, in_maps, out_maps, W1, gamma1, beta1, W2, gamma2, beta2):
    gs, ss, meta = _host_prep(np.asarray(in_maps), np.asarray(out_maps))
    key = ("v1", N, M, K, meta["TOTCOL"])
    if key not in _CACHE:
        _CACHE[key] = _build(meta)
    nc = _CACHE[key]

    feats = np.asarray(feats, np.float32)
    x16v = feats.astype(np.float16)
    w1v = np.asarray(W1, np.float32).astype(np.float16).transpose(1, 0, 2).reshape(C, K * C)
    w2v = np.asarray(W2, np.float32).astype(np.float16).transpose(1, 0, 2).reshape(C, K * C)
    bnpv = np.concatenate([np.asarray(gamma1), np.asarray(beta1),
                           np.asarray(gamma2), np.asarray(beta2)]
                          ).astype(np.float32).reshape(1, 4 * C)
    in_maps_list = []
    for c in range(NCORES):
        in_maps_list.append({
            "x16": x16v, "w1": w1v, "w2": w2v,
            "gidx": _wrap16(gs[c]), "sidx": _wrap16(ss[c]),
            "bnp": bnpv,
            "xres": x16v[c * SHARD:(c + 1) * SHARD],
        })
    return nc, in_maps_list


def kernel(feats, in_maps, out_maps, W1, gamma1, beta1, W2, gamma2, beta2):
    nc, in_maps_list = _prepare(feats, in_maps, out_maps, W1, gamma1, beta1,
                                W2, gamma2, beta2)
    from concourse.bass_utils import run_bass_kernel_spmd
    res = run_bass_kernel_spmd(nc, in_maps_list, core_ids=list(range(NCORES)))
    out = np.concatenate([res.results[c]["y"] for c in range(NCORES)], axis=0)
    return out.astype(np.float32)


# revision 15
# speedup vs baseline: 1.7792x; 1.7792x over previous
"""Trainium2 Bass kernel for nn_BasicBlock_63496796504726
(sparse 3x3x3 conv -> BN -> ReLU -> sparse conv -> BN -> +residual -> ReLU).

Sharding: out-voxel rows sharded across 8 NeuronCores (32768 rows each);
x replicated per core (fp16). Per core, kernel-map pairs whose output row
falls in its shard are processed as:
  dma_gather(transpose) per (k, in-bucket) group  -> gathered^T [C, G] fp16
  matmul (lhsT = gathered^T tile stationary, rhs = W[k])
                                                  -> contrib [128, C] psum
  copy/cast fp16 wrapped                          -> contrib [128, g, C]
  dma_scatter_add into fp16 DRAM accumulators (duplicate-free per call via
  occurrence classes; same-accumulator calls are serialized by Tile)
BN: merge accumulators, per-channel sums via ones-matmul, AllReduce [1,2C]
stats, scale/shift broadcast by rank-1 matmul, ReLU. x1 shards AllGathered
for conv2's gathers. Residual + ReLU in fp32 at the end.
"""
import sys

sys.path.insert(0, "/opt/trn_rl_repo")

import numpy as np

# problem constants (shrinkable for simulator tests)
N = 262144
C = 128
K = 27
M = 131072
NCORES = 8
SHARD = N // NCORES
BUCKET = 32768               # dma_gather int16 index window
EPS = 1e-5
NCLASS = 6                   # duplicate-occurrence classes per group
NACC = 8                     # independent accumulators per conv


def _nbucket():
    return (N + BUCKET - 1) // BUCKET


# ---------------------------------------------------------------- host prep

def _host_prep(in_maps, out_maps):
    NB = _nbucket()
    kf = np.repeat(np.arange(K), M)
    inf_ = in_maps.ravel().astype(np.int64)
    outf = out_maps.ravel().astype(np.int64)
    core = outf // SHARD
    bucket = inf_ // BUCKET
    out_local = outf % SHARD
    gloc = inf_ % BUCKET

    half = out_local // (SHARD // 2)
    order = np.lexsort((out_local, half, bucket, kf, core))
    sc, sk, sb = core[order], kf[order], bucket[order]
    sr, sg = out_local[order], gloc[order]
    sh = half[order]

    # occurrence rank within (core,k,bucket,row)
    gk = ((sc * K + sk) * NB + sb) * SHARD + sr
    new = np.empty(len(gk), bool); new[0] = True
    new[1:] = gk[1:] != gk[:-1]
    st = np.flatnonzero(new)
    occ = np.arange(len(gk)) - np.repeat(st, np.diff(np.append(st, len(gk))))
    if occ.max() >= NCLASS:
        raise RuntimeError(f"max dup occurrence {occ.max()} >= NCLASS={NCLASS}")

    counts = np.zeros((NCORES, K, NB, 2, NCLASS), np.int64)
    np.add.at(counts, (sc, sk, sb, sh, occ), 1)
    caps = counts.max(axis=0)
    caps = ((caps + 127) // 128) * 128          # 0 stays 0
    ccols = caps // 128
    gcols = ccols.sum(axis=(2, 3))
    TOTCOL = int(gcols.sum())
    TOT = TOTCOL * 128

    gofs = np.zeros((K, NB), np.int64)
    cofs = np.zeros((K, NB, 2, NCLASS), np.int64)
    acc = 0
    for k in range(K):
        for b in range(NB):
            gofs[k, b] = acc
            for h in range(2):
                for c in range(NCLASS):
                    cofs[k, b, h, c] = acc
                    acc += ccols[k, b, h, c]
    assert acc == TOTCOL

    HS = SHARD // 2
    gstr = np.zeros((NCORES, TOT), np.int16)
    sstr = np.full((NCORES, TOT), HS, np.int16)      # pads -> dump row
    pk = (((sc * K + sk) * NB + sb) * 2 + sh) * NCLASS + occ
    po = np.lexsort((np.arange(len(pk)), pk))
    pks = pk[po]
    npk = np.empty(len(pks), bool); npk[0] = True
    npk[1:] = pks[1:] != pks[:-1]
    ps = np.flatnonzero(npk)
    rank = np.arange(len(pks)) - np.repeat(ps, np.diff(np.append(ps, len(pks))))
    pos = cofs[sk[po], sb[po], sh[po], occ[po]] * 128 + rank
    gstr[sc[po], pos] = sg[po].astype(np.int16)
    sstr[sc[po], pos] = (sr[po] % HS).astype(np.int16)

    meta = dict(ccols=ccols, gcols=gcols, gofs=gofs, cofs=cofs,
                TOTCOL=TOTCOL, TOT=TOT, NB=NB)
    return gstr, sstr, meta


def _wrap16(stream):
    n = stream.shape[0]
    w = stream.reshape(n // 16, 16).T
    return np.tile(w, (8, 1)).copy()


# ---------------------------------------------------------------- builder

def _build(meta):
    import concourse.bacc as bacc
    import concourse.mybir as mybir
    import concourse.tile as tile

    f16, f32 = mybir.dt.float16, mybir.dt.float32
    i16 = mybir.dt.int16
    ccols, gcols = meta["ccols"], meta["gcols"]
    gofs, cofs = meta["gofs"], meta["cofs"]
    TOT = meta["TOT"]
    NB = meta["NB"]
    HS = SHARD // 2
    HS1 = HS + 1
    TROWS = SHARD // 128
    HROWS = HS // 128
    ZCH = min(32, HROWS)
    AACC = NACC // 2

    nc = bacc.Bacc("TRN2", target_bir_lowering=False, debug=False,
                   num_devices=NCORES)

    x16 = nc.dram_tensor("x16", [N, C], f16, kind="ExternalInput").ap()
    w1 = nc.dram_tensor("w1", [C, K * C], f16, kind="ExternalInput").ap()
    w2 = nc.dram_tensor("w2", [C, K * C], f16, kind="ExternalInput").ap()
    gidx = nc.dram_tensor("gidx", [128, TOT // 16], i16, kind="ExternalInput").ap()
    sidx = nc.dram_tensor("sidx", [128, TOT // 16], i16, kind="ExternalInput").ap()
    bnp = nc.dram_tensor("bnp", [1, 4 * C], f32, kind="ExternalInput").ap()
    xres = nc.dram_tensor("xres", [SHARD, C], f16, kind="ExternalInput").ap()
    y = nc.dram_tensor("y", [SHARD, C], f32, kind="ExternalOutput").ap()

    accs = [[[nc.dram_tensor(f"acc{cv}_{h}_{a}", [HS1, C], f16)
              for a in range(AACC)] for h in range(2)] for cv in range(2)]
    x1_shard = nc.dram_tensor("x1_shard", [SHARD, C], f16)
    x1_full = nc.dram_tensor("x1_full", [NCORES * SHARD, C], f16)
    stats_in = nc.dram_tensor("stats_in", [1, 2 * C], f32)
    stats_out = nc.dram_tensor("stats_out", [1, 2 * C], f32)

    with tile.TileContext(nc) as tc:
        with tc.tile_pool(name="persist", bufs=1) as pp:
            w_t = []
            for i in range(2):
                wti = pp.tile([C, K * C], f16, tag=f"w{i}")
                w_t.append(wti)
            nc.sync.dma_start(out=w_t[0][:], in_=w1[:])
            nc.sync.dma_start(out=w_t[1][:], in_=w2[:])
            bnp_t = pp.tile([1, 4 * C], f32)
            nc.sync.dma_start(out=bnp_t[:], in_=bnp[:])
            ones16 = pp.tile([128, 1], f16)
            nc.vector.memset(ones16[:], 1.0)
            onesr = pp.tile([1, 128], f32)
            nc.vector.memset(onesr[:], 1.0)
            zt = pp.tile([128, ZCH, C], f16)
            nc.vector.memset(zt[:], 0.0)

            for cv in range(2):
                for h in range(2):
                    for a in range(AACC):
                        av = accs[cv][h][a].ap()[0:HS, :].rearrange(
                            "(t p) c -> p t c", p=128)
                        for j in range(HROWS // ZCH):
                            nc.sync.dma_start(
                                out=av[:, j * ZCH:(j + 1) * ZCH, :], in_=zt[:])
                        nc.sync.dma_start(out=accs[cv][h][a].ap()[HS:HS1, :],
                                          in_=zt[0:1, 0, :])

            def conv(cv, src_ap):
                wt = w_t[cv]
                with tc.tile_pool(name=f"conv{cv}", bufs=4) as cp, \
                     tc.tile_pool(name=f"cps{cv}", bufs=8, space="PSUM") as cps, \
                     tc.tile_pool(name=f"cix{cv}", bufs=4) as cip:
                    call_no = 0
                    kcols = [int(gcols[k, :].sum()) for k in range(K)]
                    kmax = max(kcols) if kcols else 0
                    for k in range(K):
                        kc = kcols[k]
                        if kc == 0:
                            continue
                        k0 = int(gofs[k, 0])
                        gik = cip.tile([128, kmax * 8], i16, tag="gik")
                        sik = cip.tile([128, kmax * 8], i16, tag="sik")
                        nc.sync.dma_start(out=gik[:, 0:kc * 8],
                                          in_=gidx[:, k0 * 8:(k0 + kc) * 8])
                        nc.sync.dma_start(out=sik[:, 0:kc * 8],
                                          in_=sidx[:, k0 * 8:(k0 + kc) * 8])
                        for b in range(NB):
                            g0 = int(gofs[k, b]); gc = int(gcols[k, b])
                            if gc == 0:
                                continue
                            l0 = g0 - k0
                            gi = gik[:, l0 * 8:(l0 + gc) * 8]
                            si = sik[:, l0 * 8:(l0 + gc) * 8]
                            gt = cp.tile([C, 1, gc * 128], f16, tag="gt")
                            nc.gpsimd.dma_gather(
                                out_ap=gt[:],
                                in_ap=src_ap[b * BUCKET:(b + 1) * BUCKET, :],
                                idxs_ap=gi,
                                num_idxs=gc * 128,
                                num_idxs_reg=gc * 128,
                                elem_size=C,
                                transpose=True,
                            )
                            contrib = cp.tile([128, gc, C], f16, tag="ct")
                            for gidx_cp, t0 in enumerate(range(0, gc, 4)):
                                tw = min(4, gc - t0)
                                pst = cps.tile([128, 4 * C], f32, tag="ps")
                                for t in range(t0, t0 + tw):
                                    nc.tensor.matmul(
                                        pst[:, (t - t0) * C:(t - t0 + 1) * C],
                                        lhsT=gt[:, 0, t * 128:(t + 1) * 128],
                                        rhs=wt[:, k * C:(k + 1) * C],
                                        start=True, stop=True)
                                eng = nc.scalar if (gidx_cp % 2 == 0) else nc.vector
                                if gidx_cp % 2 == 0:
                                    nc.scalar.copy(
                                        contrib[:, t0:t0 + tw, :],
                                        pst[:, 0:tw * C].rearrange(
                                            "p (t c) -> p t c", c=C))
                                else:
                                    nc.vector.tensor_copy(
                                        contrib[:, t0:t0 + tw, :],
                                        pst[:, 0:tw * C].rearrange(
                                            "p (t c) -> p t c", c=C))
                            for hh in range(2):
                              for cl in range(NCLASS):
                                cc = int(ccols[k, b, hh, cl])
                                if cc == 0:
                                    continue
                                c0 = int(cofs[k, b, hh, cl]) - g0
                                acc_t = accs[cv][hh][call_no % AACC]
                                call_no += 1
                                nc.gpsimd.dma_scatter_add(
                                    out_ap=acc_t.ap(),
                                    in_ap=contrib[:, c0:c0 + cc, :],
                                    idxs_ap=si[:, c0 * 8:(c0 + cc) * 8],
                                    num_idxs=cc * 128,
                                    num_idxs_reg=cc * 128,
                                    elem_size=C)

            def merge_bn(cv, out_cb):
                with tc.tile_pool(name=f"bn{cv}", bufs=1) as bp, \
                     tc.tile_pool(name=f"bna{cv}", bufs=4) as bap, \
                     tc.tile_pool(name=f"bnp{cv}", bufs=1, space="PSUM") as bps:
                    msb = bp.tile([128, TROWS, C], f16, tag="msb")
                    pss = bps.tile([1, 512], f32, tag="pss")
                    psq = bps.tile([1, 512], f32, tag="psq")
                    nch = TROWS // 4
                    for j in range(nch):
                        hh = j // (nch // 2)
                        jj = j % (nch // 2)
                        mt = bap.tile([128, 4, C], f16, tag="mt")
                        av0 = accs[cv][hh][0].ap()[0:HS, :].rearrange(
                            "(t p) c -> p t c", p=128)
                        nc.sync.dma_start(out=mt[:], in_=av0[:, jj * 4:(jj + 1) * 4, :])
                        for a in range(1, AACC):
                            at = bap.tile([128, 4, C], f16, tag="at")
                            av = accs[cv][hh][a].ap()[0:HS, :].rearrange(
                                "(t p) c -> p t c", p=128)
                            nc.sync.dma_start(out=at[:],
                                              in_=av[:, jj * 4:(jj + 1) * 4, :])
                            nc.vector.tensor_add(mt[:], mt[:], at[:])
                        nc.vector.tensor_copy(msb[:, j * 4:(j + 1) * 4, :], mt[:])
                        sq = bap.tile([128, 4, C], f16, tag="sq")
                        nc.vector.tensor_mul(sq[:], mt[:], mt[:])
                        nc.tensor.matmul(
                            pss[:], lhsT=ones16[:],
                            rhs=mt[:].rearrange("p t c -> p (t c)"),
                            start=(j == 0), stop=(j == nch - 1))
                        nc.tensor.matmul(
                            psq[:], lhsT=ones16[:],
                            rhs=sq[:].rearrange("p t c -> p (t c)"),
                            start=(j == 0), stop=(j == nch - 1))
                    pssb = bp.tile([1, 512], f32, tag="pssb")
                    psqb = bp.tile([1, 512], f32, tag="psqb")
                    nc.vector.tensor_copy(pssb[:], pss[:])
                    nc.vector.tensor_copy(psqb[:], psq[:])
                    stt = bp.tile([1, 2 * C], f32, tag="stt")
                    nc.vector.tensor_add(stt[:, 0:C], pssb[:, 0:C], pssb[:, C:2 * C])
                    nc.vector.tensor_add(stt[:, 0:C], stt[:, 0:C], pssb[:, 2 * C:3 * C])
                    nc.vector.tensor_add(stt[:, 0:C], stt[:, 0:C], pssb[:, 3 * C:4 * C])
                    nc.vector.tensor_add(stt[:, C:2 * C], psqb[:, 0:C], psqb[:, C:2 * C])
                    nc.vector.tensor_add(stt[:, C:2 * C], stt[:, C:2 * C],
                                         psqb[:, 2 * C:3 * C])
                    nc.vector.tensor_add(stt[:, C:2 * C], stt[:, C:2 * C],
                                         psqb[:, 3 * C:4 * C])
                    nc.sync.dma_start(out=stats_in.ap(), in_=stt[:])
                    nc.gpsimd.collective_compute(
                        "AllReduce", mybir.AluOpType.add,
                        replica_groups=[list(range(NCORES))],
                        ins=[stats_in.ap().opt()],
                        outs=[stats_out.ap().opt()])
                    stg = bp.tile([1, 2 * C], f32, tag="stg")
                    nc.sync.dma_start(out=stg[:], in_=stats_out.ap())
                    mean = bp.tile([1, C], f32, tag="mean")
                    var = bp.tile([1, C], f32, tag="var")
                    inv = bp.tile([1, C], f32, tag="inv")
                    msq = bp.tile([1, C], f32, tag="msq")
                    sc_t = bp.tile([1, C], f32, tag="sct")
                    sh_t = bp.tile([1, C], f32, tag="sht")
                    nc.vector.tensor_scalar_mul(mean[:], stg[:, 0:C], 1.0 / N)
                    nc.vector.tensor_scalar_mul(var[:], stg[:, C:2 * C], 1.0 / N)
                    nc.vector.tensor_mul(msq[:], mean[:], mean[:])
                    nc.vector.tensor_sub(var[:], var[:], msq[:])
                    nc.vector.tensor_scalar_add(var[:], var[:], EPS)
                    nc.scalar.activation(inv[:], var[:],
                                         mybir.ActivationFunctionType.Sqrt)
                    nc.vector.reciprocal(inv[:], inv[:])
                    nc.vector.tensor_mul(sc_t[:], inv[:],
                                         bnp_t[:, 2 * cv * C:(2 * cv + 1) * C])
                    nc.vector.tensor_mul(sh_t[:], mean[:], sc_t[:])
                    nc.vector.tensor_sub(sh_t[:], bnp_t[:, (2 * cv + 1) * C:(2 * cv + 2) * C],
                                         sh_t[:])
                    psb = bps.tile([128, 2 * C], f32, tag="psb")
                    nc.tensor.matmul(psb[:, 0:C], lhsT=onesr[:], rhs=sc_t[:],
                                     start=True, stop=True)
                    nc.tensor.matmul(psb[:, C:2 * C], lhsT=onesr[:], rhs=sh_t[:],
                                     start=True, stop=True)
                    scb = bp.tile([128, C], f16, tag="scb")
                    shb = bp.tile([128, C], f16, tag="shb")
                    nc.vector.tensor_copy(scb[:], psb[:, 0:C])
                    nc.vector.tensor_copy(shb[:], psb[:, C:2 * C])
                    for j in range(nch):
                        nt = bap.tile([128, 4, C], f16, tag="nt")
                        for tt in range(4):
                            nc.vector.tensor_mul(
                                nt[:, tt, :], msb[:, j * 4 + tt, :], scb[:])
                            nc.vector.tensor_add(
                                nt[:, tt, :], nt[:, tt, :], shb[:])
                        out_cb(j, nt, bap)

            conv(0, x16)

            x1v = x1_shard.ap().rearrange("(t p) c -> p t c", p=128)

            def bn1_out(j, nt, bap):
                xt = bap.tile([128, 4, C], f16, tag="xt")
                nc.vector.tensor_scalar_max(xt[:], nt[:], 0.0)
                nc.sync.dma_start(out=x1v[:, j * 4:(j + 1) * 4, :], in_=xt[:])

            merge_bn(0, bn1_out)

            import concourse.mybir as mybir2
            nc.gpsimd.collective_compute(
                "AllGather", mybir2.AluOpType.bypass,
                replica_groups=[list(range(NCORES))],
                ins=[x1_shard.ap().opt()],
                outs=[x1_full.ap().opt()])

            conv(1, x1_full.ap())

            yv = y.rearrange("(t p) c -> p t c", p=128)
            xrv = xres.rearrange("(t p) c -> p t c", p=128)

            def bn2_out(j, nt, bap):
                rt = bap.tile([128, 4, C], f32, tag="rt")
                rt16 = bap.tile([128, 4, C], f16, tag="rt16")
                nc.sync.dma_start(out=rt16[:],
                                  in_=xrv[:, j * 4:(j + 1) * 4, :])
                nc.vector.tensor_copy(rt[:], rt16[:])
                nt32 = bap.tile([128, 4, C], f32, tag="nt32")
                nc.vector.tensor_copy(nt32[:], nt[:])
                nc.vector.tensor_add(nt32[:], nt32[:], rt[:])
                yt = bap.tile([128, 4, C], f32, tag="yt")
                nc.vector.tensor_scalar_max(yt[:], nt32[:], 0.0)
                nc.sync.dma_start(out=yv[:, j * 4:(j + 1) * 4, :], in_=yt[:])

            merge_bn(1, bn2_out)

    nc.compile()
    return nc


_CACHE = {}


def _prepare(feats, in_maps, out_maps, W1, gamma1, beta1, W2, gamma2, beta2):
    gs, ss, meta = _host_prep(np.asarray(in_maps), np.asarray(out_maps))
    key = ("v1", N, M, K, meta["TOTCOL"])
    if key not in _CACHE:
        _CACHE[key] = _build(meta)
    nc = _CACHE[key]

    feats = np.asarray(feats, np.float32)
    x16v = feats.astype(np.float16)
    w1v = np.asarray(W1, np.float32).astype(np.float16).transpose(1, 0, 2).reshape(C, K * C)
    w2v = np.asarray(W2, np.float32).astype(np.float16).transpose(1, 0, 2).reshape(C, K * C)
    bnpv = np.concatenate([np.asarray(gamma1), np.asarray(beta1),
                           np.asarray(gamma2), np.asarray(beta2)]
                          ).astype(np.float32).reshape(1, 4 * C)
    in_maps_list = []
    for c in range(NCORES):
        in_maps_list.append({
            "x16": x16v, "w1": w1v, "w2": w2v,
            "gidx": _wrap16(gs[c]), "sidx": _wrap16(ss[c]),
            "bnp": bnpv,
            "xres": x16v[c * SHARD:(c + 1) * SHARD],
        })
    return nc, in_maps_list


def kernel(feats, in_maps, out_maps, W1, gamma1, beta1, W2, gamma2, beta2):
    nc, in_maps_list = _prepare(feats, in_maps, out_maps, W1, gamma1, beta1,
                                W2, gamma2, beta2)
    from concourse.bass_utils import run_bass_kernel_spmd
    res = run_bass_kernel_spmd(nc, in_maps_list, core_ids=list(range(NCORES)))
    out = np.concatenate([res.results[c]["y"] for c in range(NCORES)], axis=0)
    return out.astype(np.float32)
